# revision 32
# baseline (speedup 1.0000x reference)
"""CAMoE-GNN Trainium2 kernel (8 NeuronCores, SPMD).

Math (reference, per layer):
    gate = softmax((top @ Wg.T)/TEMP)            [N,3]
    he   = h @ W[e]
    agg  = segsum(he[src]*dinv[src]*dinv[dst] -> dst)   (incl. self loops)
    out  = sum_e gate_e * relu(agg_e + b[e])

Key algebra used here:
    aggregation commutes with W[e]:  agg_e = (A @ h) @ W[e]  with
    A = D^-1/2 (M + I) D^-1/2.  So the sparse phase runs ONCE per layer:
        hagg_raw[d] = sum_{(s,d)} dinv_s * h[s]      (0/1 selection matmuls)
    and the dense phase applies, per node chunk (128 rows):
        P_e   = hagg_raw @ W_e + sqrt(deg) x b_e     (rank-1 bias, PE k=1 mm)
        out_e = relu(P_e * (gate_e * dinv_d^p))      (per-partition scale)
    where p=2 for layer 1 (folds the pre-scale of the next layer's gather
    source: we exchange hs1 = dinv*h1) and p=1 for layer 2.

Sharding: nodes are relabeled so each core owns 6250 nodes arranged into 49
windows of 128 "slots"; relabeling greedily balances sum(deg) per window.
Each core aggregates the in-edges of its own nodes.

Layer 1 streams host-pre-gathered fp8 tokens + fp8 0/1 selection matrices,
both partition-major [128, totch, F] (one large contiguous-per-partition
DMA per 4-window group); the PE pairs chunks with fp8 DoubleRow perf mode.
hs1 is exchanged in TWO AllGather pieces split at window 24: piece 0 fires
mid-way through the layer-1 dense loop and piece 1 at its end, so piece-0
exchange hides under dense compute and layer-2 bucket-A gathers (which
read only piece 0) overlap the piece-1 exchange.  Layer 2 gathers bf16
rows with one large dma_gather per bucket per group (~35 chunks/call).
The token buckets A/B are the two pieces (both piece row spaces fit int16).
"""

import os
import numpy as np
import ml_dtypes

N = 50000
E = 800000
F = 128
HID = 128
OUT = 64
TOP = 4
EXP = 3
G = 64
TEMP = 101.0
W_CORES = 8
NSH = N // W_CORES          # 6250 nodes per core
WPC = 49                    # windows per core (48*128 + 106)
WSLOT = 128
NPAD = WPC * WSLOT          # 6272 padded local nodes
WSPLIT = 30                 # piece 0 = windows [0, 30), piece 1 = [30, 49)
PA = WSPLIT * WSLOT         # 3072 rows per core in piece 0
PB = NSH - PA               # 3178 rows per core in piece 1
GROUPS = [tuple(range(w, w + 4)) for w in range(0, 48, 4)] + [(48,)]


# ----------------------------------------------------------------- host plan


def _build_plan(edge_index, batch):
    src = np.asarray(edge_index[0], dtype=np.int64)
    dst = np.asarray(edge_index[1], dtype=np.int64)
    sl = np.arange(N, dtype=np.int64)
    s_all = np.concatenate([src, sl])
    d_all = np.concatenate([dst, sl])
    deg = np.bincount(d_all, minlength=N).astype(np.float64)  # includes self
    dinv = 1.0 / np.sqrt(deg)

    # --- relabel: greedy balance of sum(deg) over 8*49 windows (cap 128/106)
    order = np.argsort(-deg, kind="stable")
    nbins = W_CORES * WPC
    caps = np.full(nbins, WSLOT, np.int64)
    caps[WPC - 1 :: WPC] = NSH - 48 * WSLOT  # last window per core: 106
    load = np.zeros(nbins, np.float64)
    fill = np.zeros(nbins, np.int64)
    import heapq

    heap = [(0.0, int(b)) for b in range(nbins)]
    heapq.heapify(heap)
    binof = np.empty(N, np.int64)
    posof = np.empty(N, np.int64)
    for nid in order:
        while True:
            l, b = heapq.heappop(heap)
            if fill[b] < caps[b]:
                break
        binof[nid] = b
        posof[nid] = fill[b]
        fill[b] += 1
        load[b] = l + deg[nid]
        if fill[b] < caps[b]:
            heapq.heappush(heap, (load[b], b))
    c_of_bin = binof // WPC
    w_of_bin = binof % WPC
    new_id = c_of_bin * NSH + w_of_bin * WSLOT + posof

    ns = new_id[s_all]
    nd = new_id[d_all]
    core = nd // NSH
    loc = nd % NSH
    win = loc // WSLOT
    slot = loc % WSLOT

    # source row in piece-local coordinates
    s_core = ns // NSH
    s_loc = ns % NSH
    in_a = s_loc < PA
    rowA = s_core * PA + s_loc             # valid where in_a
    rowB = s_core * PB + (s_loc - PA)      # valid where ~in_a

    RA = np.zeros(WPC, np.int64)
    RB = np.zeros(WPC, np.int64)
    tokA = {}
    tokB = {}
    okey = core * WPC + win
    osort = np.argsort(okey, kind="stable")
    ns_a, slot_s, okey_s = in_a[osort], slot[osort], okey[osort]
    rowA_s, rowB_s = rowA[osort], rowB[osort]
    bounds = np.searchsorted(okey_s, np.arange(W_CORES * WPC + 1))
    nA = np.zeros((W_CORES, WPC), np.int64)
    nB = np.zeros((W_CORES, WPC), np.int64)
    for c in range(W_CORES):
        for w in range(WPC):
            k = c * WPC + w
            seg = slice(bounds[k], bounds[k + 1])
            fa = ns_a[seg]
            tokA[(c, w)] = (rowA_s[seg][fa], slot_s[seg][fa])
            tokB[(c, w)] = (rowB_s[seg][~fa], slot_s[seg][~fa])
            nA[c, w] = int(fa.sum())
            nB[c, w] = int((~fa).sum())
    for w in range(WPC):
        RA[w] = max(1, int(np.ceil(nA[:, w].max() / WSLOT)))
        RB[w] = max(1, int(np.ceil(nB[:, w].max() / WSLOT)))

    # chunk storage order per group g: [w0 A][w1 A].. | [w0 B][w1 B]..
    totch = int(sum((RA[w] + RB[w]) for w in range(WPC)))
    idx_np = np.zeros((W_CORES, 128, totch * 8), np.int16)
    selT_np = np.zeros((W_CORES, 128, totch, 128), ml_dtypes.float8_e4m3)
    tok_srcA = np.full((W_CORES, totch, 128), -1, np.int64)  # piece-A rows
    tok_srcB = np.full((W_CORES, totch, 128), -1, np.int64)  # piece-B rows
    ch_base_A = {}
    ch_base_B = {}
    ch = 0
    for grp in GROUPS:
        for w in grp:
            ch_base_A[w] = ch
            ch += int(RA[w])
        for w in grp:
            ch_base_B[w] = ch
            ch += int(RB[w])
    assert ch == totch

    def fill_tokens(c, w, ch0, nch, s_arr, l_arr, srcbuf):
        n = len(s_arr)
        assert n <= nch * WSLOT
        iv = s_arr.astype(np.int16)
        t = np.arange(n)
        chv = ch0 + t // WSLOT
        pv = t % WSLOT
        selT_np[c, pv, chv, l_arr] = 1.0
        srcbuf[c, chv, pv] = s_arr
        # idx wrapped layout per chunk: token p at [p%16, chunk*8 + p//16]
        cols = chv * 8 + pv // 16
        rows = pv % 16
        idx_np[c, rows, cols] = iv

    for c in range(W_CORES):
        for w in range(WPC):
            sa, la = tokA[(c, w)]
            fill_tokens(c, w, ch_base_A[w], int(RA[w]), sa, la, tok_srcA)
            sb, lb = tokB[(c, w)]
            fill_tokens(c, w, ch_base_B[w], int(RB[w]), sb, lb, tok_srcB)
    # replicate idx pattern across the 8 groups of 16 partitions
    idx_np[:, 16:, :] = np.tile(idx_np[:, :16, :], (1, 7, 1))

    # per-core node-level arrays in relabeled order
    inv = np.empty(N, np.int64)
    inv[new_id] = np.arange(N)

    nb = np.asarray(batch, dtype=np.int64)
    cnt = np.bincount(nb, minlength=G).astype(np.float64)

    plan = {
        "deg": deg,
        "dinv": dinv,
        "new_id": new_id,
        "inv": inv,
        "RA": RA,
        "RB": RB,
        "totch": totch,
        "idx": idx_np,
        "selT": selT_np,
        "cnt": cnt,
        "batch_new": nb[inv],  # graph id per relabeled node
        "tok_srcA": tok_srcA,
        "tok_srcB": tok_srcB,
    }
    return plan


# ------------------------------------------------------------- device build


def _build_nc(RA, RB, totch):
    import concourse.bacc as bacc
    import concourse.mybir as mybir
    import concourse.tile as tile

    fp32 = mybir.dt.float32
    bf16 = mybir.dt.bfloat16
    fp8 = mybir.dt.float8e4
    i16 = mybir.dt.int16
    DR = mybir.MatmulPerfMode.DoubleRow

    nc = bacc.Bacc("TRN2", debug=False, num_swdge_queues=4)

    tok0 = nc.dram_tensor("tok0", [128, totch, F], fp8, kind="ExternalInput")
    idxs = nc.dram_tensor("idxs", [128, totch * 8], i16, kind="ExternalInput")
    sels = nc.dram_tensor("sels", [128, totch, 128], fp8, kind="ExternalInput")
    wall0 = nc.dram_tensor("wall0", [F, EXP * HID], bf16, kind="ExternalInput")
    wall1 = nc.dram_tensor("wall1", [F, EXP * HID], bf16, kind="ExternalInput")
    ball0 = nc.dram_tensor("ball0", [1, EXP * HID], bf16, kind="ExternalInput")
    ball1 = nc.dram_tensor("ball1", [1, EXP * HID], bf16, kind="ExternalInput")
    sqdeg = nc.dram_tensor("sqdeg", [1, NPAD], bf16, kind="ExternalInput")
    dpow1 = nc.dram_tensor("dpow1", [128, WPC], fp32, kind="ExternalInput")
    dpow2 = nc.dram_tensor("dpow2", [128, WPC], fp32, kind="ExternalInput")
    topt = nc.dram_tensor("topt", [TOP, NPAD], bf16, kind="ExternalInput")
    wgt0 = nc.dram_tensor("wgt0", [TOP, EXP], bf16, kind="ExternalInput")
    wgt1 = nc.dram_tensor("wgt1", [TOP, EXP], bf16, kind="ExternalInput")
    h2out = nc.dram_tensor("h2out", [NSH, HID], bf16, kind="ExternalOutput")

    shard_a = nc.dram_tensor("shard_a", [PA, F], bf16)
    shard_b = nc.dram_tensor("shard_b", [PB, F], bf16)
    full1a = nc.dram_tensor("full1a", [W_CORES * PA, F], bf16,
                            addr_space="Shared")
    full1b = nc.dram_tensor("full1b", [W_CORES * PB, F], bf16,
                            addr_space="Shared")

    with tile.TileContext(nc) as tc:
        with tc.tile_pool(name="persist", bufs=1) as pp, \
             tc.tile_pool(name="wt", bufs=1) as wtp, \
             tc.tile_pool(name="stream", bufs=2) as sp, \
             tc.tile_pool(name="chunks", bufs=3) as cp, \
             tc.tile_pool(name="gatha", bufs=5) as gpa, \
             tc.tile_pool(name="gathb", bufs=3) as gpb, \
             tc.tile_pool(name="psum", bufs=4, space="PSUM") as ps, \
             tc.tile_pool(name="psume", bufs=3, space="PSUM") as pse:

            # ---------- resident data
            idx_sb = pp.tile([128, totch * 8], i16)
            nc.sync.dma_start(out=idx_sb[:], in_=idxs[:])
            hagg = pp.tile([128, NPAD], bf16)          # haggT, f-major
            sq_sb = pp.tile([1, NPAD], bf16)
            nc.sync.dma_start(out=sq_sb[:], in_=sqdeg[:])
            topt_sb = pp.tile([TOP, NPAD], bf16)
            nc.sync.dma_start(out=topt_sb[:], in_=topt[:])
            w_sb = [wtp.tile([F, EXP * HID], bf16, tag=f"w{l}", name=f"w{l}") for l in range(2)]
            nc.sync.dma_start(out=w_sb[0][:], in_=wall0[:])
            nc.sync.dma_start(out=w_sb[1][:], in_=wall1[:])
            b_sb = [wtp.tile([1, EXP * HID], bf16, tag=f"b{l}", name=f"b{l}") for l in range(2)]
            nc.sync.dma_start(out=b_sb[0][:], in_=ball0[:])
            nc.sync.dma_start(out=b_sb[1][:], in_=ball1[:])
            wg_sb = [wtp.tile([TOP, EXP], bf16, tag=f"wg{l}", name=f"wg{l}") for l in range(2)]
            nc.sync.dma_start(out=wg_sb[0][:], in_=wgt0[:])
            nc.sync.dma_start(out=wg_sb[1][:], in_=wgt1[:])
            dp_sb = [wtp.tile([128, WPC], fp32, tag=f"dp{l}", name=f"dp{l}") for l in range(2)]
            nc.sync.dma_start(out=dp_sb[0][:], in_=dpow1[:])
            nc.sync.dma_start(out=dp_sb[1][:], in_=dpow2[:])

            # chunk offsets in storage/call order
            chA, chB = {}, {}
            ch = 0
            for grp in GROUPS:
                for w in grp:
                    chA[w] = ch
                    ch += int(RA[w])
                for w in grp:
                    chB[w] = ch
                    ch += int(RB[w])

            scale_sb = []

            def compute_gates():
                # gate scale columns for both layers: [128, WPC, EXP]
                for l in range(2):
                    glog = pp.tile([128, WPC, EXP], fp32, tag=f"glog{l}", name=f"glog{l}")
                    for k in range(WPC):
                        pg = ps.tile([128, EXP], fp32, space="PSUM", tag="pw")
                        nc.tensor.matmul(
                            out=pg[:],
                            lhsT=topt_sb[:, k * 128 : (k + 1) * 128],
                            rhs=wg_sb[l][:],
                            start=True, stop=True,
                        )
                        nc.vector.tensor_copy(out=glog[:, k, :], in_=pg[:])
                    gexp = pp.tile([128, WPC, EXP], fp32, tag=f"gexp{l}", name=f"gexp{l}")
                    nc.scalar.activation(gexp[:], glog[:],
                                         mybir.ActivationFunctionType.Exp,
                                         bias=0.0, scale=1.0 / TEMP)
                    gsum = pp.tile([128, WPC], fp32, tag=f"gsum{l}", name=f"gsum{l}")
                    nc.vector.tensor_reduce(out=gsum[:], in_=gexp[:],
                                            axis=mybir.AxisListType.X,
                                            op=mybir.AluOpType.add)
                    grec = pp.tile([128, WPC], fp32, tag=f"grec{l}", name=f"grec{l}")
                    nc.vector.reciprocal(out=grec[:], in_=gsum[:])
                    rd = pp.tile([128, WPC], fp32, tag=f"rd{l}", name=f"rd{l}")
                    nc.vector.tensor_mul(out=rd[:], in0=grec[:], in1=dp_sb[l][:])
                    sc = pp.tile([128, WPC, EXP], fp32, tag=f"sc{l}", name=f"sc{l}")
                    for e in range(EXP):
                        nc.vector.tensor_mul(out=sc[:, :, e], in0=gexp[:, :, e],
                                             in1=rd[:])
                    scale_sb.append(sc)

            qrr = [0]

            def gather_calls(gtile, src_ap, ch0, nch):
                # 8-chunk single-packet SWDGE calls, round-robin queues
                off = 0
                while off < nch:
                    n = min(8, nch - off)
                    nc.gpsimd.dma_gather(
                        gtile[:, off : off + n, :], src_ap,
                        idx_sb[:, (ch0 + off) * 8 : (ch0 + off + n) * 8],
                        n * 128, n * 128, F, single_packet=True,
                        queue_num=qrr[0] % 4)
                    qrr[0] += 1
                    off += n

            PF = 4  # A-bucket gather prefetch depth (groups)

            def issue_gA(grp):
                ra = sum(int(RA[w]) for w in grp)
                gA = gpa.tile([128, ra, F], bf16, tag="gA")
                gather_calls(gA, full1a[:], chA[grp[0]], ra)
                return gA

            def sparse_and_dense(l, store_l1):
                if l == 0:
                    compute_gates()
                gA_pend = {}
                gB0 = None
                if l == 1:
                    # issue one piece-1 gather first: it waits for the
                    # second AllGather, serializing every later gather
                    # behind it on the gpsimd queue so the collectives
                    # run without DMA contention.
                    grp0 = GROUPS[0]
                    rb0 = sum(int(RB[w]) for w in grp0)
                    gB0 = gpb.tile([128, rb0, F], bf16, tag="gB")
                    gather_calls(gB0, full1b[:], chB[grp0[0]], rb0)
                    for gi in range(min(PF, len(GROUPS))):
                        gA_pend[gi] = issue_gA(GROUPS[gi])
                for gidx, grp in enumerate(GROUPS):
                    ra = sum(int(RA[w]) for w in grp)
                    rb = sum(int(RB[w]) for w in grp)
                    c0 = chA[grp[0]]
                    cb0 = chB[grp[0]]
                    selAll = cp.tile([128, ra + rb, 128], fp8, tag="selAll")
                    nc.sync.dma_start(
                        out=selAll[:], in_=sels[:, c0 : c0 + ra + rb, :])
                    if l == 0:
                        gAll = cp.tile([128, ra + rb, F], fp8, tag="gAll")
                        nc.sync.dma_start(
                            out=gAll[:], in_=tok0[:, c0 : c0 + ra + rb, :])
                        gA = gAll
                        gB = gAll
                        boff = 0          # B chunks at [ra, ra+rb) in gAll
                    else:
                        gA = gA_pend.pop(gidx)
                        if gidx + PF < len(GROUPS):
                            gA_pend[gidx + PF] = issue_gA(GROUPS[gidx + PF])
                        if gidx == 0:
                            gB = gB0
                        else:
                            gB = gpb.tile([128, rb, F], bf16, tag="gB")
                            gather_calls(gB, full1b[:], cb0, rb)
                        boff = -ra        # gB chunk axis starts at 0
                    a_off = 0
                    b_off = 0
                    for w in grp:
                        pw = ps.tile([128, WSLOT], fp32, space="PSUM", tag="pw")
                        # runs: (token tile, tile_off - sel_off, sel chunk, count)
                        nmm_plan = []
                        for tile_, toff, selc, cnt in [
                            (gA, 0, a_off, int(RA[w])),
                            (gB, boff, ra + b_off, int(RB[w])),
                        ]:
                            r = 0
                            while r < cnt:
                                if l == 0 and r + 1 < cnt:
                                    nmm_plan.append((tile_, toff, selc + r, 2))
                                    r += 2
                                else:
                                    nmm_plan.append((tile_, toff, selc + r, 1))
                                    r += 1
                        nmm = len(nmm_plan)
                        for j, (tile_, toff, selc, k) in enumerate(nmm_plan):
                            tc0 = selc + toff
                            if k == 2:
                                nc.tensor.matmul(
                                    out=pw[:],
                                    lhsT=tile_[:, tc0 : tc0 + 2, :],
                                    rhs=selAll[:, selc : selc + 2, :],
                                    start=(j == 0), stop=(j == nmm - 1),
                                    perf_mode=DR)
                            else:
                                nc.tensor.matmul(
                                    out=pw[:],
                                    lhsT=tile_[:, tc0, :],
                                    rhs=selAll[:, selc, :],
                                    start=(j == 0), stop=(j == nmm - 1))
                        a_off += int(RA[w])
                        b_off += int(RB[w])
                        nc.vector.tensor_copy(
                            out=hagg[:, w * 128 : (w + 1) * 128], in_=pw[:])
                    # dense for this group's windows, interleaved so the
                    # PE fills gather-wait gaps and (layer 1) the piece-0
                    # AllGather fires mid-way through the sparse stream
                    for k in grp:
                        dense_window(l, k, store_l1)

            def dense_window(l, k, store_l1):
                pe = pse.tile([128, EXP * HID], fp32, space="PSUM", tag="pe")
                nc.tensor.matmul(
                    out=pe[:], lhsT=hagg[:, k * 128 : (k + 1) * 128],
                    rhs=w_sb[l][:], start=True, stop=False)
                nc.tensor.matmul(
                    out=pe[:], lhsT=sq_sb[:, k * 128 : (k + 1) * 128],
                    rhs=b_sb[l][:], start=False, stop=True)
                # experts 0,1 on the scalar engine; expert 2 on DVE
                aex = []
                for e in range(2):
                    a = cp.tile([128, HID], fp32, tag=f"a{e}", name=f"a{e}")
                    nc.scalar.activation(
                        a[:], pe[:, e * HID : (e + 1) * HID],
                        mybir.ActivationFunctionType.Relu,
                        bias=0.0, scale=scale_sb[l][:, k, e : e + 1])
                    aex.append(a)
                a2 = cp.tile([128, HID], fp32, tag="a2", name="a2")
                nc.vector.tensor_scalar(
                    out=a2[:], in0=pe[:, 2 * HID : 3 * HID],
                    scalar1=scale_sb[l][:, k, 2 : 3], scalar2=0.0,
                    op0=mybir.AluOpType.mult, op1=mybir.AluOpType.max)
                hout = cp.tile([128, HID], fp32, tag="hout")
                nc.vector.tensor_add(out=hout[:], in0=aex[0][:], in1=aex[1][:])
                hbf = cp.tile([128, HID], bf16, tag="hbf")
                nc.vector.tensor_add(out=hbf[:], in0=hout[:], in1=a2[:])
                rows = min(128, NSH - k * 128)
                if store_l1:
                    if k < WSPLIT:
                        nc.sync.dma_start(
                            out=shard_a[k * 128 : k * 128 + rows, :],
                            in_=hbf[:rows, :])
                    else:
                        r0 = (k - WSPLIT) * 128
                        nc.sync.dma_start(
                            out=shard_b[r0 : r0 + rows, :],
                            in_=hbf[:rows, :])
                    if k == WSPLIT - 1:
                        nc.gpsimd.collective_compute(
                            "AllGather", mybir.AluOpType.bypass,
                            ins=[shard_a[:]], outs=[full1a[:]],
                            replica_groups=[list(range(W_CORES))])
                else:
                    nc.sync.dma_start(
                        out=h2out[k * 128 : k * 128 + rows, :],
                        in_=hbf[:rows, :])

            # ---------- layer 1 (piece-0 AllGather fires inside its dense loop)
            sparse_and_dense(0, store_l1=True)
            nc.gpsimd.collective_compute(
                "AllGather", mybir.AluOpType.bypass,
                ins=[shard_b[:]], outs=[full1b[:]],
                replica_groups=[list(range(W_CORES))])
            # ---------- layer 2 (h2 rows stream out; pooling on host)
            sparse_and_dense(1, store_l1=False)

    nc.compile()
    return nc


# ------------------------------------------------------------------- kernel


def kernel(**inputs):
    x = np.asarray(inputs["x"], np.float32)
    top_features = np.asarray(inputs["top_features"], np.float32)
    edge_index = np.asarray(inputs["edge_index"])
    batch = np.asarray(inputs["batch"])
    W0 = np.asarray(inputs["W0"], np.float32)
    b0 = np.asarray(inputs["b0"], np.float32)
    Wg0 = np.asarray(inputs["Wg0"], np.float32)
    W1 = np.asarray(inputs["W1"], np.float32)
    b1 = np.asarray(inputs["b1"], np.float32)
    Wg1 = np.asarray(inputs["Wg1"], np.float32)
    Wf = np.asarray(inputs["Wf"], np.float32)
    bf = np.asarray(inputs["bf"], np.float32)

    plan = _build_plan(edge_index, batch)
    dinv = plan["dinv"]
    inv = plan["inv"]          # relabeled -> original node id
    RA, RB, totch = plan["RA"], plan["RB"], plan["totch"]

    # gather source (layer 1): x * dinv, relabeled order, fp8
    xs8 = (x * dinv[:, None])[inv].astype(ml_dtypes.float8_e4m3)

    deg_new = plan["deg"][inv]
    dinv_new = dinv[inv]
    top_new = top_features[inv]
    batch_new = plan["batch_new"]

    def pad_npad(a):
        out = np.zeros((W_CORES, NPAD) + a.shape[1:], a.dtype)
        for c in range(W_CORES):
            out[c, : 48 * WSLOT] = a[c * NSH : c * NSH + 48 * WSLOT]
            # last window: 106 real slots
            out[c, 48 * WSLOT : 48 * WSLOT + (NSH - 48 * WSLOT)] = \
                a[c * NSH + 48 * WSLOT : (c + 1) * NSH]
        return out

    sq_pad = pad_npad(np.sqrt(deg_new).astype(np.float32))       # [8, NPAD]
    d1_pad = pad_npad((dinv_new ** 2).astype(np.float32))
    d2_pad = pad_npad(dinv_new.astype(np.float32))
    top_pad = pad_npad(top_new.astype(np.float32))               # [8,NPAD,4]
    bat_pad = pad_npad(batch_new)
    # mark pad slots: zero scales, selg zero
    padmask = pad_npad(np.ones(N, np.float32))

    d1_pad *= padmask
    d2_pad *= padmask

    wall0 = W0.transpose(1, 0, 2).reshape(F, EXP * HID).copy()
    wall1 = W1.transpose(1, 0, 2).reshape(F, EXP * HID).copy()
    ball0 = b0.reshape(1, EXP * HID).copy()
    ball1 = b1.reshape(1, EXP * HID).copy()

    # relabeled-node -> tok0 source row (both pieces share xs8 order)
    # piece-local row -> relabeled id: A: c*PA + loc ; B: c*PB + loc - ...
    in_maps = []
    for c in range(W_CORES):
        tsA = plan["tok_srcA"][c]
        tsB = plan["tok_srcB"][c]
        tok0_c = np.zeros((totch, 128, F), ml_dtypes.float8_e4m3)
        va = tsA >= 0
        if va.any():
            ra = tsA[va]
            rel = (ra // PA) * NSH + (ra % PA)
            tok0_c[va] = xs8[rel]
        vb = tsB >= 0
        if vb.any():
            rb = tsB[vb]
            rel = (rb // PB) * NSH + PA + (rb % PB)
            tok0_c[vb] = xs8[rel]
        tok0T_c = np.ascontiguousarray(tok0_c.transpose(1, 0, 2))
        in_maps.append({
            "tok0": tok0T_c,
            "idxs": plan["idx"][c],
            "sels": np.ascontiguousarray(plan["selT"][c]),
            "wall0": wall0.astype(ml_dtypes.bfloat16),
            "wall1": wall1.astype(ml_dtypes.bfloat16),
            "ball0": ball0.astype(ml_dtypes.bfloat16),
            "ball1": ball1.astype(ml_dtypes.bfloat16),
            "sqdeg": sq_pad[c][None, :].astype(ml_dtypes.bfloat16),
            "dpow1": d1_pad[c].reshape(WPC, 128).T.copy(),
            "dpow2": d2_pad[c].reshape(WPC, 128).T.copy(),
            "topt": top_pad[c].T.copy().astype(ml_dtypes.bfloat16),
            "wgt0": Wg0.T.copy().astype(ml_dtypes.bfloat16),
            "wgt1": Wg1.T.copy().astype(ml_dtypes.bfloat16),
        })

    from concourse.bass_utils import run_bass_kernel_spmd

    nc = _build_nc(RA, RB, totch)
    trace = os.environ.get("KERNEL_TRACE", "0") == "1"
    ncores = int(os.environ.get("KERNEL_CORES", str(W_CORES)))
    res = run_bass_kernel_spmd(nc, in_maps[:ncores], core_ids=list(range(ncores)),
                               trace=trace)
    kernel.last_results = res

    # host-side pooling: segment-mean over graphs + final linear layer
    h2 = np.concatenate(
        [np.asarray(res.results[c]["h2out"]).astype(np.float64)
         for c in range(W_CORES)], axis=0)                    # [N, HID] relab
    sums = np.zeros((G, HID), np.float64)
    np.add.at(sums, batch_new, h2)
    cnt = np.maximum(plan["cnt"], 1.0)
    pooled = sums / cnt[:, None]
    out = pooled @ Wf.astype(np.float64) + bf.astype(np.float64)[None, :]
    return out.astype(np.float32)


# revision 34
# speedup vs baseline: 1.1005x; 1.1005x over previous
"""CAMoE-GNN Trainium2 kernel (8 NeuronCores, SPMD).

Math (reference, per layer):
    gate = softmax((top @ Wg.T)/TEMP)            [N,3]
    he   = h @ W[e]
    agg  = segsum(he[src]*dinv[src]*dinv[dst] -> dst)   (incl. self loops)
    out  = sum_e gate_e * relu(agg_e + b[e])

Key algebra used here:
    aggregation commutes with W[e]:  agg_e = (A @ h) @ W[e]  with
    A = D^-1/2 (M + I) D^-1/2.  So the sparse phase runs ONCE per layer:
        hagg_raw[d] = sum_{(s,d)} dinv_s * h[s]      (0/1 selection matmuls)
    and the dense phase applies, per node chunk (128 rows):
        P_e   = hagg_raw @ W_e + sqrt(deg) x b_e     (rank-1 bias, PE k=1 mm)
        out_e = relu(P_e * (gate_e * dinv_d^p))      (per-partition scale)
    where p=2 for layer 1 (folds the pre-scale of the next layer's gather
    source: we exchange hs1 = dinv*h1) and p=1 for layer 2.

Sharding: nodes are relabeled so each core owns 6250 nodes arranged into 49
windows of 128 "slots"; relabeling greedily balances sum(deg) per window.
Each core aggregates the in-edges of its own nodes.

Layer 1 streams host-pre-gathered fp8 tokens + fp8 0/1 selection matrices,
both partition-major [128, totch, F] (one large contiguous-per-partition
DMA per 4-window group); the PE pairs chunks with fp8 DoubleRow perf mode.
hs1 is exchanged in TWO AllGather pieces split at window 24: piece 0 fires
mid-way through the layer-1 dense loop and piece 1 at its end, so piece-0
exchange hides under dense compute and layer-2 bucket-A gathers (which
read only piece 0) overlap the piece-1 exchange.  Layer 2 gathers bf16
rows with one large dma_gather per bucket per group (~35 chunks/call).
The token buckets A/B are the two pieces (both piece row spaces fit int16).
"""

import os
import numpy as np
import ml_dtypes

N = 50000
E = 800000
F = 128
HID = 128
OUT = 64
TOP = 4
EXP = 3
G = 64
TEMP = 101.0
W_CORES = 8
NSH = N // W_CORES          # 6250 nodes per core
WPC = 49                    # windows per core (48*128 + 106)
WSLOT = 128
NPAD = WPC * WSLOT          # 6272 padded local nodes
WSPLIT = 32                 # piece 0 = windows [0, 32), piece 1 = [32, 49)
PA = WSPLIT * WSLOT         # 3072 rows per core in piece 0
PB = NSH - PA               # 3178 rows per core in piece 1
GROUPS = [tuple(range(w, w + 4)) for w in range(0, 48, 4)] + [(48,)]


# ----------------------------------------------------------------- host plan


def _build_plan(edge_index, batch):
    src = np.asarray(edge_index[0], dtype=np.int64)
    dst = np.asarray(edge_index[1], dtype=np.int64)
    sl = np.arange(N, dtype=np.int64)
    s_all = np.concatenate([src, sl])
    d_all = np.concatenate([dst, sl])
    deg = np.bincount(d_all, minlength=N).astype(np.float64)  # includes self
    dinv = 1.0 / np.sqrt(deg)

    # --- relabel: greedy balance of sum(deg) over 8*49 windows (cap 128/106)
    order = np.argsort(-deg, kind="stable")
    nbins = W_CORES * WPC
    caps = np.full(nbins, WSLOT, np.int64)
    caps[WPC - 1 :: WPC] = NSH - 48 * WSLOT  # last window per core: 106
    load = np.zeros(nbins, np.float64)
    fill = np.zeros(nbins, np.int64)
    import heapq

    heap = [(0.0, int(b)) for b in range(nbins)]
    heapq.heapify(heap)
    binof = np.empty(N, np.int64)
    posof = np.empty(N, np.int64)
    for nid in order:
        while True:
            l, b = heapq.heappop(heap)
            if fill[b] < caps[b]:
                break
        binof[nid] = b
        posof[nid] = fill[b]
        fill[b] += 1
        load[b] = l + deg[nid]
        if fill[b] < caps[b]:
            heapq.heappush(heap, (load[b], b))
    c_of_bin = binof // WPC
    w_of_bin = binof % WPC
    new_id = c_of_bin * NSH + w_of_bin * WSLOT + posof

    ns = new_id[s_all]
    nd = new_id[d_all]
    core = nd // NSH
    loc = nd % NSH
    win = loc // WSLOT
    slot = loc % WSLOT

    # source row in piece-local coordinates
    s_core = ns // NSH
    s_loc = ns % NSH
    in_a = s_loc < PA
    rowA = s_core * PA + s_loc             # valid where in_a
    rowB = s_core * PB + (s_loc - PA)      # valid where ~in_a

    RA = np.zeros(WPC, np.int64)
    RB = np.zeros(WPC, np.int64)
    tokA = {}
    tokB = {}
    okey = core * WPC + win
    osort = np.argsort(okey, kind="stable")
    ns_a, slot_s, okey_s = in_a[osort], slot[osort], okey[osort]
    rowA_s, rowB_s = rowA[osort], rowB[osort]
    bounds = np.searchsorted(okey_s, np.arange(W_CORES * WPC + 1))
    nA = np.zeros((W_CORES, WPC), np.int64)
    nB = np.zeros((W_CORES, WPC), np.int64)
    for c in range(W_CORES):
        for w in range(WPC):
            k = c * WPC + w
            seg = slice(bounds[k], bounds[k + 1])
            fa = ns_a[seg]
            tokA[(c, w)] = (rowA_s[seg][fa], slot_s[seg][fa])
            tokB[(c, w)] = (rowB_s[seg][~fa], slot_s[seg][~fa])
            nA[c, w] = int(fa.sum())
            nB[c, w] = int((~fa).sum())
    for w in range(WPC):
        RA[w] = max(1, int(np.ceil(nA[:, w].max() / WSLOT)))
        RB[w] = max(1, int(np.ceil(nB[:, w].max() / WSLOT)))

    # chunk storage order per group g: [w0 A][w1 A].. | [w0 B][w1 B]..
    totch = int(sum((RA[w] + RB[w]) for w in range(WPC)))
    idx_np = np.zeros((W_CORES, 128, totch * 8), np.int16)
    selT_np = np.zeros((W_CORES, 128, totch, 128), ml_dtypes.float8_e4m3)
    tok_srcA = np.full((W_CORES, totch, 128), -1, np.int64)  # piece-A rows
    tok_srcB = np.full((W_CORES, totch, 128), -1, np.int64)  # piece-B rows
    ch_base_A = {}
    ch_base_B = {}
    ch = 0
    for grp in GROUPS:
        for w in grp:
            ch_base_A[w] = ch
            ch += int(RA[w])
        for w in grp:
            ch_base_B[w] = ch
            ch += int(RB[w])
    assert ch == totch

    def fill_tokens(c, w, ch0, nch, s_arr, l_arr, srcbuf):
        n = len(s_arr)
        assert n <= nch * WSLOT
        iv = s_arr.astype(np.int16)
        t = np.arange(n)
        chv = ch0 + t // WSLOT
        pv = t % WSLOT
        selT_np[c, pv, chv, l_arr] = 1.0
        srcbuf[c, chv, pv] = s_arr
        # idx wrapped layout per chunk: token p at [p%16, chunk*8 + p//16]
        cols = chv * 8 + pv // 16
        rows = pv % 16
        idx_np[c, rows, cols] = iv

    for c in range(W_CORES):
        for w in range(WPC):
            sa, la = tokA[(c, w)]
            fill_tokens(c, w, ch_base_A[w], int(RA[w]), sa, la, tok_srcA)
            sb, lb = tokB[(c, w)]
            fill_tokens(c, w, ch_base_B[w], int(RB[w]), sb, lb, tok_srcB)
    # replicate idx pattern across the 8 groups of 16 partitions
    idx_np[:, 16:, :] = np.tile(idx_np[:, :16, :], (1, 7, 1))

    # per-core node-level arrays in relabeled order
    inv = np.empty(N, np.int64)
    inv[new_id] = np.arange(N)

    nb = np.asarray(batch, dtype=np.int64)
    cnt = np.bincount(nb, minlength=G).astype(np.float64)

    plan = {
        "deg": deg,
        "dinv": dinv,
        "new_id": new_id,
        "inv": inv,
        "RA": RA,
        "RB": RB,
        "totch": totch,
        "idx": idx_np,
        "selT": selT_np,
        "cnt": cnt,
        "batch_new": nb[inv],  # graph id per relabeled node
        "tok_srcA": tok_srcA,
        "tok_srcB": tok_srcB,
    }
    return plan


# ------------------------------------------------------------- device build


def _build_nc(RA, RB, totch):
    import concourse.bacc as bacc
    import concourse.mybir as mybir
    import concourse.tile as tile

    fp32 = mybir.dt.float32
    bf16 = mybir.dt.bfloat16
    fp8 = mybir.dt.float8e4
    i16 = mybir.dt.int16
    DR = mybir.MatmulPerfMode.DoubleRow

    nc = bacc.Bacc("TRN2", debug=False, num_swdge_queues=4)

    tok0 = nc.dram_tensor("tok0", [128, totch, F], fp8, kind="ExternalInput")
    idxs = nc.dram_tensor("idxs", [128, totch * 8], i16, kind="ExternalInput")
    sels = nc.dram_tensor("sels", [128, totch, 128], fp8, kind="ExternalInput")
    wall0 = nc.dram_tensor("wall0", [F, EXP * HID], bf16, kind="ExternalInput")
    wall1 = nc.dram_tensor("wall1", [F, EXP * HID], bf16, kind="ExternalInput")
    ball0 = nc.dram_tensor("ball0", [1, EXP * HID], bf16, kind="ExternalInput")
    ball1 = nc.dram_tensor("ball1", [1, EXP * HID], bf16, kind="ExternalInput")
    sqdeg = nc.dram_tensor("sqdeg", [1, NPAD], bf16, kind="ExternalInput")
    dpow1 = nc.dram_tensor("dpow1", [128, WPC], fp32, kind="ExternalInput")
    dpow2 = nc.dram_tensor("dpow2", [128, WPC], fp32, kind="ExternalInput")
    topt = nc.dram_tensor("topt", [TOP, NPAD], bf16, kind="ExternalInput")
    wgt0 = nc.dram_tensor("wgt0", [TOP, EXP], bf16, kind="ExternalInput")
    wgt1 = nc.dram_tensor("wgt1", [TOP, EXP], bf16, kind="ExternalInput")
    h2out = nc.dram_tensor("h2out", [NSH, HID], bf16, kind="ExternalOutput")

    shard_a = nc.dram_tensor("shard_a", [PA, F], bf16)
    shard_b = nc.dram_tensor("shard_b", [PB, F], bf16)
    full1a = nc.dram_tensor("full1a", [W_CORES * PA, F], bf16,
                            addr_space="Shared")
    full1b = nc.dram_tensor("full1b", [W_CORES * PB, F], bf16,
                            addr_space="Shared")

    with tile.TileContext(nc) as tc:
        with tc.tile_pool(name="persist", bufs=1) as pp, \
             tc.tile_pool(name="wt", bufs=1) as wtp, \
             tc.tile_pool(name="stream", bufs=2) as sp, \
             tc.tile_pool(name="chunks", bufs=3) as cp, \
             tc.tile_pool(name="gatha", bufs=5) as gpa, \
             tc.tile_pool(name="gathb", bufs=4) as gpb, \
             tc.tile_pool(name="psum", bufs=4, space="PSUM") as ps, \
             tc.tile_pool(name="psume", bufs=3, space="PSUM") as pse:

            # ---------- resident data (gate inputs first so the PE can
            # start on the gate matmuls immediately; the 1.8MB idx table
            # is only needed by layer-2 gathers, load it last)
            topt_sb = pp.tile([TOP, NPAD], bf16)
            nc.sync.dma_start(out=topt_sb[:], in_=topt[:])
            hagg = pp.tile([128, NPAD], bf16)          # haggT, f-major
            sq_sb = pp.tile([1, NPAD], bf16)
            nc.sync.dma_start(out=sq_sb[:], in_=sqdeg[:])
            w_sb = [wtp.tile([F, EXP * HID], bf16, tag=f"w{l}", name=f"w{l}") for l in range(2)]
            nc.sync.dma_start(out=w_sb[0][:], in_=wall0[:])
            nc.sync.dma_start(out=w_sb[1][:], in_=wall1[:])
            b_sb = [wtp.tile([1, EXP * HID], bf16, tag=f"b{l}", name=f"b{l}") for l in range(2)]
            nc.sync.dma_start(out=b_sb[0][:], in_=ball0[:])
            nc.sync.dma_start(out=b_sb[1][:], in_=ball1[:])
            wg_sb = [wtp.tile([TOP, EXP], bf16, tag=f"wg{l}", name=f"wg{l}") for l in range(2)]
            nc.sync.dma_start(out=wg_sb[0][:], in_=wgt0[:])
            nc.sync.dma_start(out=wg_sb[1][:], in_=wgt1[:])
            dp_sb = [wtp.tile([128, WPC], fp32, tag=f"dp{l}", name=f"dp{l}") for l in range(2)]
            nc.sync.dma_start(out=dp_sb[0][:], in_=dpow1[:])
            nc.sync.dma_start(out=dp_sb[1][:], in_=dpow2[:])
            idx_sb = pp.tile([128, totch * 8], i16)
            nc.sync.dma_start(out=idx_sb[:], in_=idxs[:])

            # chunk offsets in storage/call order
            chA, chB = {}, {}
            ch = 0
            for grp in GROUPS:
                for w in grp:
                    chA[w] = ch
                    ch += int(RA[w])
                for w in grp:
                    chB[w] = ch
                    ch += int(RB[w])

            scale_sb = []

            def compute_gates():
                # gate scale columns for both layers: [128, WPC, EXP]
                for l in range(2):
                    glog = pp.tile([128, WPC, EXP], fp32, tag=f"glog{l}", name=f"glog{l}")
                    for k in range(WPC):
                        pg = ps.tile([128, EXP], fp32, space="PSUM", tag="pw")
                        nc.tensor.matmul(
                            out=pg[:],
                            lhsT=topt_sb[:, k * 128 : (k + 1) * 128],
                            rhs=wg_sb[l][:],
                            start=True, stop=True,
                        )
                        nc.vector.tensor_copy(out=glog[:, k, :], in_=pg[:])
                    gexp = pp.tile([128, WPC, EXP], fp32, tag=f"gexp{l}", name=f"gexp{l}")
                    nc.scalar.activation(gexp[:], glog[:],
                                         mybir.ActivationFunctionType.Exp,
                                         bias=0.0, scale=1.0 / TEMP)
                    gsum = pp.tile([128, WPC], fp32, tag=f"gsum{l}", name=f"gsum{l}")
                    nc.vector.tensor_reduce(out=gsum[:], in_=gexp[:],
                                            axis=mybir.AxisListType.X,
                                            op=mybir.AluOpType.add)
                    grec = pp.tile([128, WPC], fp32, tag=f"grec{l}", name=f"grec{l}")
                    nc.vector.reciprocal(out=grec[:], in_=gsum[:])
                    rd = pp.tile([128, WPC], fp32, tag=f"rd{l}", name=f"rd{l}")
                    nc.vector.tensor_mul(out=rd[:], in0=grec[:], in1=dp_sb[l][:])
                    sc = pp.tile([128, WPC, EXP], fp32, tag=f"sc{l}", name=f"sc{l}")
                    for e in range(EXP):
                        nc.vector.tensor_mul(out=sc[:, :, e], in0=gexp[:, :, e],
                                             in1=rd[:])
                    scale_sb.append(sc)

            qrr = [0]

            def gather_calls(gtile, src_ap, ch0, nch):
                # 8-chunk single-packet SWDGE calls, round-robin queues
                off = 0
                while off < nch:
                    n = min(8, nch - off)
                    nc.gpsimd.dma_gather(
                        gtile[:, off : off + n, :], src_ap,
                        idx_sb[:, (ch0 + off) * 8 : (ch0 + off + n) * 8],
                        n * 128, n * 128, F, single_packet=True,
                        queue_num=qrr[0] % 4)
                    qrr[0] += 1
                    off += n

            PF = 4  # A-bucket gather prefetch depth (groups)

            def issue_gA(grp):
                ra = sum(int(RA[w]) for w in grp)
                gA = gpa.tile([128, ra, F], bf16, tag="gA")
                gather_calls(gA, full1a[:], chA[grp[0]], ra)
                return gA

            def sparse_and_dense(l, store_l1):
                if l == 0:
                    compute_gates()
                gA_pend = {}
                gB0 = None
                if l == 1:
                    # issue one piece-1 gather first: it waits for the
                    # second AllGather, serializing every later gather
                    # behind it on the gpsimd queue so the collectives
                    # run without DMA contention.
                    grp0 = GROUPS[0]
                    rb0 = sum(int(RB[w]) for w in grp0)
                    gB0 = gpb.tile([128, rb0, F], bf16, tag="gB")
                    gather_calls(gB0, full1b[:], chB[grp0[0]], rb0)
                    for gi in range(min(PF, len(GROUPS))):
                        gA_pend[gi] = issue_gA(GROUPS[gi])
                for gidx, grp in enumerate(GROUPS):
                    ra = sum(int(RA[w]) for w in grp)
                    rb = sum(int(RB[w]) for w in grp)
                    c0 = chA[grp[0]]
                    cb0 = chB[grp[0]]
                    selAll = cp.tile([128, ra + rb, 128], fp8, tag="selAll")
                    nc.sync.dma_start(
                        out=selAll[:], in_=sels[:, c0 : c0 + ra + rb, :])
                    if l == 0:
                        gAll = cp.tile([128, ra + rb, F], fp8, tag="gAll")
                        nc.sync.dma_start(
                            out=gAll[:], in_=tok0[:, c0 : c0 + ra + rb, :])
                        gA = gAll
                        gB = gAll
                        boff = 0          # B chunks at [ra, ra+rb) in gAll
                    else:
                        gA = gA_pend.pop(gidx)
                        if gidx + PF < len(GROUPS):
                            gA_pend[gidx + PF] = issue_gA(GROUPS[gidx + PF])
                        if gidx == 0:
                            gB = gB0
                        else:
                            gB = gpb.tile([128, rb, F], bf16, tag="gB")
                            gather_calls(gB, full1b[:], cb0, rb)
                        boff = -ra        # gB chunk axis starts at 0
                    a_off = 0
                    b_off = 0
                    for w in grp:
                        pw = ps.tile([128, WSLOT], fp32, space="PSUM", tag="pw")
                        # runs: (token tile, tile_off - sel_off, sel chunk, count)
                        nmm_plan = []
                        for tile_, toff, selc, cnt in [
                            (gA, 0, a_off, int(RA[w])),
                            (gB, boff, ra + b_off, int(RB[w])),
                        ]:
                            r = 0
                            while r < cnt:
                                if l == 0 and r + 1 < cnt:
                                    nmm_plan.append((tile_, toff, selc + r, 2))
                                    r += 2
                                else:
                                    nmm_plan.append((tile_, toff, selc + r, 1))
                                    r += 1
                        nmm = len(nmm_plan)
                        for j, (tile_, toff, selc, k) in enumerate(nmm_plan):
                            tc0 = selc + toff
                            if k == 2:
                                nc.tensor.matmul(
                                    out=pw[:],
                                    lhsT=tile_[:, tc0 : tc0 + 2, :],
                                    rhs=selAll[:, selc : selc + 2, :],
                                    start=(j == 0), stop=(j == nmm - 1),
                                    perf_mode=DR)
                            else:
                                nc.tensor.matmul(
                                    out=pw[:],
                                    lhsT=tile_[:, tc0, :],
                                    rhs=selAll[:, selc, :],
                                    start=(j == 0), stop=(j == nmm - 1))
                        a_off += int(RA[w])
                        b_off += int(RB[w])
                        nc.vector.tensor_copy(
                            out=hagg[:, w * 128 : (w + 1) * 128], in_=pw[:])
                    # dense for this group's windows, interleaved so the
                    # PE fills gather-wait gaps and (layer 1) the piece-0
                    # AllGather fires mid-way through the sparse stream
                    for k in grp:
                        dense_window(l, k, store_l1)

            def dense_window(l, k, store_l1):
                pe = pse.tile([128, EXP * HID], fp32, space="PSUM", tag="pe")
                nc.tensor.matmul(
                    out=pe[:], lhsT=hagg[:, k * 128 : (k + 1) * 128],
                    rhs=w_sb[l][:], start=True, stop=False)
                nc.tensor.matmul(
                    out=pe[:], lhsT=sq_sb[:, k * 128 : (k + 1) * 128],
                    rhs=b_sb[l][:], start=False, stop=True)
                # experts 0,1 on the scalar engine; expert 2 on DVE
                aex = []
                for e in range(2):
                    a = cp.tile([128, HID], fp32, tag=f"a{e}", name=f"a{e}")
                    nc.scalar.activation(
                        a[:], pe[:, e * HID : (e + 1) * HID],
                        mybir.ActivationFunctionType.Relu,
                        bias=0.0, scale=scale_sb[l][:, k, e : e + 1])
                    aex.append(a)
                a2 = cp.tile([128, HID], fp32, tag="a2", name="a2")
                nc.vector.tensor_scalar(
                    out=a2[:], in0=pe[:, 2 * HID : 3 * HID],
                    scalar1=scale_sb[l][:, k, 2 : 3], scalar2=0.0,
                    op0=mybir.AluOpType.mult, op1=mybir.AluOpType.max)
                hout = cp.tile([128, HID], fp32, tag="hout")
                nc.vector.tensor_add(out=hout[:], in0=aex[0][:], in1=aex[1][:])
                hbf = cp.tile([128, HID], bf16, tag="hbf")
                nc.vector.tensor_add(out=hbf[:], in0=hout[:], in1=a2[:])
                rows = min(128, NSH - k * 128)
                if store_l1:
                    if k < WSPLIT:
                        nc.sync.dma_start(
                            out=shard_a[k * 128 : k * 128 + rows, :],
                            in_=hbf[:rows, :])
                    else:
                        r0 = (k - WSPLIT) * 128
                        nc.sync.dma_start(
                            out=shard_b[r0 : r0 + rows, :],
                            in_=hbf[:rows, :])
                    if k == WSPLIT - 1:
                        nc.gpsimd.collective_compute(
                            "AllGather", mybir.AluOpType.bypass,
                            ins=[shard_a[:]], outs=[full1a[:]],
                            replica_groups=[list(range(W_CORES))])
                else:
                    nc.sync.dma_start(
                        out=h2out[k * 128 : k * 128 + rows, :],
                        in_=hbf[:rows, :])

            # ---------- layer 1 (piece-0 AllGather fires inside its dense loop)
            sparse_and_dense(0, store_l1=True)
            nc.gpsimd.collective_compute(
                "AllGather", mybir.AluOpType.bypass,
                ins=[shard_b[:]], outs=[full1b[:]],
                replica_groups=[list(range(W_CORES))])
            # ---------- layer 2 (h2 rows stream out; pooling on host)
            sparse_and_dense(1, store_l1=False)

    nc.compile()
    return nc


# ------------------------------------------------------------------- kernel


def kernel(**inputs):
    x = np.asarray(inputs["x"], np.float32)
    top_features = np.asarray(inputs["top_features"], np.float32)
    edge_index = np.asarray(inputs["edge_index"])
    batch = np.asarray(inputs["batch"])
    W0 = np.asarray(inputs["W0"], np.float32)
    b0 = np.asarray(inputs["b0"], np.float32)
    Wg0 = np.asarray(inputs["Wg0"], np.float32)
    W1 = np.asarray(inputs["W1"], np.float32)
    b1 = np.asarray(inputs["b1"], np.float32)
    Wg1 = np.asarray(inputs["Wg1"], np.float32)
    Wf = np.asarray(inputs["Wf"], np.float32)
    bf = np.asarray(inputs["bf"], np.float32)

    plan = _build_plan(edge_index, batch)
    dinv = plan["dinv"]
    inv = plan["inv"]          # relabeled -> original node id
    RA, RB, totch = plan["RA"], plan["RB"], plan["totch"]

    # gather source (layer 1): x * dinv, relabeled order, fp8
    xs8 = (x * dinv[:, None])[inv].astype(ml_dtypes.float8_e4m3)

    deg_new = plan["deg"][inv]
    dinv_new = dinv[inv]
    top_new = top_features[inv]
    batch_new = plan["batch_new"]

    def pad_npad(a):
        out = np.zeros((W_CORES, NPAD) + a.shape[1:], a.dtype)
        for c in range(W_CORES):
            out[c, : 48 * WSLOT] = a[c * NSH : c * NSH + 48 * WSLOT]
            # last window: 106 real slots
            out[c, 48 * WSLOT : 48 * WSLOT + (NSH - 48 * WSLOT)] = \
                a[c * NSH + 48 * WSLOT : (c + 1) * NSH]
        return out

    sq_pad = pad_npad(np.sqrt(deg_new).astype(np.float32))       # [8, NPAD]
    d1_pad = pad_npad((dinv_new ** 2).astype(np.float32))
    d2_pad = pad_npad(dinv_new.astype(np.float32))
    top_pad = pad_npad(top_new.astype(np.float32))               # [8,NPAD,4]
    bat_pad = pad_npad(batch_new)
    # mark pad slots: zero scales, selg zero
    padmask = pad_npad(np.ones(N, np.float32))

    d1_pad *= padmask
    d2_pad *= padmask

    wall0 = W0.transpose(1, 0, 2).reshape(F, EXP * HID).copy()
    wall1 = W1.transpose(1, 0, 2).reshape(F, EXP * HID).copy()
    ball0 = b0.reshape(1, EXP * HID).copy()
    ball1 = b1.reshape(1, EXP * HID).copy()

    # relabeled-node -> tok0 source row (both pieces share xs8 order)
    # piece-local row -> relabeled id: A: c*PA + loc ; B: c*PB + loc - ...
    in_maps = []
    for c in range(W_CORES):
        tsA = plan["tok_srcA"][c]
        tsB = plan["tok_srcB"][c]
        tok0_c = np.zeros((totch, 128, F), ml_dtypes.float8_e4m3)
        va = tsA >= 0
        if va.any():
            ra = tsA[va]
            rel = (ra // PA) * NSH + (ra % PA)
            tok0_c[va] = xs8[rel]
        vb = tsB >= 0
        if vb.any():
            rb = tsB[vb]
            rel = (rb // PB) * NSH + PA + (rb % PB)
            tok0_c[vb] = xs8[rel]
        tok0T_c = np.ascontiguousarray(tok0_c.transpose(1, 0, 2))
        in_maps.append({
            "tok0": tok0T_c,
            "idxs": plan["idx"][c],
            "sels": np.ascontiguousarray(plan["selT"][c]),
            "wall0": wall0.astype(ml_dtypes.bfloat16),
            "wall1": wall1.astype(ml_dtypes.bfloat16),
            "ball0": ball0.astype(ml_dtypes.bfloat16),
            "ball1": ball1.astype(ml_dtypes.bfloat16),
            "sqdeg": sq_pad[c][None, :].astype(ml_dtypes.bfloat16),
            "dpow1": d1_pad[c].reshape(WPC, 128).T.copy(),
            "dpow2": d2_pad[c].reshape(WPC, 128).T.copy(),
            "topt": top_pad[c].T.copy().astype(ml_dtypes.bfloat16),
            "wgt0": Wg0.T.copy().astype(ml_dtypes.bfloat16),
            "wgt1": Wg1.T.copy().astype(ml_dtypes.bfloat16),
        })

    from concourse.bass_utils import run_bass_kernel_spmd

    nc = _build_nc(RA, RB, totch)
    trace = os.environ.get("KERNEL_TRACE", "0") == "1"
    ncores = int(os.environ.get("KERNEL_CORES", str(W_CORES)))
    res = run_bass_kernel_spmd(nc, in_maps[:ncores], core_ids=list(range(ncores)),
                               trace=trace)
    kernel.last_results = res

    # host-side pooling: segment-mean over graphs + final linear layer
    h2 = np.concatenate(
        [np.asarray(res.results[c]["h2out"]).astype(np.float64)
         for c in range(W_CORES)], axis=0)                    # [N, HID] relab
    sums = np.zeros((G, HID), np.float64)
    np.add.at(sums, batch_new, h2)
    cnt = np.maximum(plan["cnt"], 1.0)
    pooled = sums / cnt[:, None]
    out = pooled @ Wf.astype(np.float64) + bf.astype(np.float64)[None, :]
    return out.astype(np.float32)


# revision 36
# speedup vs baseline: 1.1176x; 1.0155x over previous
"""CAMoE-GNN Trainium2 kernel (8 NeuronCores, SPMD).

Math (reference, per layer):
    gate = softmax((top @ Wg.T)/TEMP)            [N,3]
    he   = h @ W[e]
    agg  = segsum(he[src]*dinv[src]*dinv[dst] -> dst)   (incl. self loops)
    out  = sum_e gate_e * relu(agg_e + b[e])

Key algebra used here:
    aggregation commutes with W[e]:  agg_e = (A @ h) @ W[e]  with
    A = D^-1/2 (M + I) D^-1/2.  So the sparse phase runs ONCE per layer:
        hagg_raw[d] = sum_{(s,d)} dinv_s * h[s]      (0/1 selection matmuls)
    and the dense phase applies, per node chunk (128 rows):
        P_e   = hagg_raw @ W_e + sqrt(deg) x b_e     (rank-1 bias, PE k=1 mm)
        out_e = relu(P_e * (gate_e * dinv_d^p))      (per-partition scale)
    where p=2 for layer 1 (folds the pre-scale of the next layer's gather
    source: we exchange hs1 = dinv*h1) and p=1 for layer 2.

Sharding: nodes are relabeled so each core owns 6250 nodes arranged into 49
windows of 128 "slots"; relabeling greedily balances sum(deg) per window.
Each core aggregates the in-edges of its own nodes.

Layer 1 streams host-pre-gathered fp8 tokens + fp8 0/1 selection matrices,
both partition-major [128, totch, F] (one large contiguous-per-partition
DMA per 4-window group); the PE pairs chunks with fp8 DoubleRow perf mode.
hs1 is exchanged in TWO AllGather pieces split at window 24: piece 0 fires
mid-way through the layer-1 dense loop and piece 1 at its end, so piece-0
exchange hides under dense compute and layer-2 bucket-A gathers (which
read only piece 0) overlap the piece-1 exchange.  Layer 2 gathers bf16
rows with one large dma_gather per bucket per group (~35 chunks/call).
The token buckets A/B are the two pieces (both piece row spaces fit int16).
"""

import os
import numpy as np
import ml_dtypes

N = 50000
E = 800000
F = 128
HID = 128
OUT = 64
TOP = 4
EXP = 3
G = 64
TEMP = 101.0
W_CORES = 8
NSH = N // W_CORES          # 6250 nodes per core
WPC = 49                    # windows per core (48*128 + 106)
WSLOT = 128
NPAD = WPC * WSLOT          # 6272 padded local nodes
WSPLIT = 32                 # piece 0 = windows [0, 32), piece 1 = [32, 49)
PA = WSPLIT * WSLOT         # 3072 rows per core in piece 0
PB = NSH - PA               # 3178 rows per core in piece 1
GROUPS = [tuple(range(w, w + 4)) for w in range(0, 48, 4)] + [(48,)]


# ----------------------------------------------------------------- host plan


def _build_plan(edge_index, batch):
    src = np.asarray(edge_index[0], dtype=np.int64)
    dst = np.asarray(edge_index[1], dtype=np.int64)
    sl = np.arange(N, dtype=np.int64)
    s_all = np.concatenate([src, sl])
    d_all = np.concatenate([dst, sl])
    deg = np.bincount(d_all, minlength=N).astype(np.float64)  # includes self
    dinv = 1.0 / np.sqrt(deg)

    # --- relabel: greedy balance of sum(deg) over 8*49 windows (cap 128/106)
    order = np.argsort(-deg, kind="stable")
    nbins = W_CORES * WPC
    caps = np.full(nbins, WSLOT, np.int64)
    caps[WPC - 1 :: WPC] = NSH - 48 * WSLOT  # last window per core: 106
    load = np.zeros(nbins, np.float64)
    fill = np.zeros(nbins, np.int64)
    import heapq

    heap = [(0.0, int(b)) for b in range(nbins)]
    heapq.heapify(heap)
    binof = np.empty(N, np.int64)
    posof = np.empty(N, np.int64)
    for nid in order:
        while True:
            l, b = heapq.heappop(heap)
            if fill[b] < caps[b]:
                break
        binof[nid] = b
        posof[nid] = fill[b]
        fill[b] += 1
        load[b] = l + deg[nid]
        if fill[b] < caps[b]:
            heapq.heappush(heap, (load[b], b))
    c_of_bin = binof // WPC
    w_of_bin = binof % WPC
    new_id = c_of_bin * NSH + w_of_bin * WSLOT + posof

    ns = new_id[s_all]
    nd = new_id[d_all]
    core = nd // NSH
    loc = nd % NSH
    win = loc // WSLOT
    slot = loc % WSLOT

    # source row in piece-local coordinates
    s_core = ns // NSH
    s_loc = ns % NSH
    in_a = s_loc < PA
    rowA = s_core * PA + s_loc             # valid where in_a
    rowB = s_core * PB + (s_loc - PA)      # valid where ~in_a

    RA = np.zeros(WPC, np.int64)
    RB = np.zeros(WPC, np.int64)
    tokA = {}
    tokB = {}
    okey = core * WPC + win
    osort = np.argsort(okey, kind="stable")
    ns_a, slot_s, okey_s = in_a[osort], slot[osort], okey[osort]
    rowA_s, rowB_s = rowA[osort], rowB[osort]
    bounds = np.searchsorted(okey_s, np.arange(W_CORES * WPC + 1))
    nA = np.zeros((W_CORES, WPC), np.int64)
    nB = np.zeros((W_CORES, WPC), np.int64)
    for c in range(W_CORES):
        for w in range(WPC):
            k = c * WPC + w
            seg = slice(bounds[k], bounds[k + 1])
            fa = ns_a[seg]
            tokA[(c, w)] = (rowA_s[seg][fa], slot_s[seg][fa])
            tokB[(c, w)] = (rowB_s[seg][~fa], slot_s[seg][~fa])
            nA[c, w] = int(fa.sum())
            nB[c, w] = int((~fa).sum())
    for w in range(WPC):
        RA[w] = max(1, int(np.ceil(nA[:, w].max() / WSLOT)))
        RB[w] = max(1, int(np.ceil(nB[:, w].max() / WSLOT)))

    # chunk storage order per group g: [w0 A][w1 A].. | [w0 B][w1 B]..
    totch = int(sum((RA[w] + RB[w]) for w in range(WPC)))
    idx_np = np.zeros((W_CORES, 128, totch * 8), np.int16)
    selT_np = np.zeros((W_CORES, 128, totch, 128), ml_dtypes.float8_e4m3)
    tok_srcA = np.full((W_CORES, totch, 128), -1, np.int64)  # piece-A rows
    tok_srcB = np.full((W_CORES, totch, 128), -1, np.int64)  # piece-B rows
    ch_base_A = {}
    ch_base_B = {}
    ch = 0
    for grp in GROUPS:
        for w in grp:
            ch_base_A[w] = ch
            ch += int(RA[w])
        for w in grp:
            ch_base_B[w] = ch
            ch += int(RB[w])
    assert ch == totch

    def fill_tokens(c, w, ch0, nch, s_arr, l_arr, srcbuf):
        n = len(s_arr)
        assert n <= nch * WSLOT
        iv = s_arr.astype(np.int16)
        t = np.arange(n)
        chv = ch0 + t // WSLOT
        pv = t % WSLOT
        selT_np[c, pv, chv, l_arr] = 1.0
        srcbuf[c, chv, pv] = s_arr
        # idx wrapped layout per chunk: token p at [p%16, chunk*8 + p//16]
        cols = chv * 8 + pv // 16
        rows = pv % 16
        idx_np[c, rows, cols] = iv

    for c in range(W_CORES):
        for w in range(WPC):
            sa, la = tokA[(c, w)]
            fill_tokens(c, w, ch_base_A[w], int(RA[w]), sa, la, tok_srcA)
            sb, lb = tokB[(c, w)]
            fill_tokens(c, w, ch_base_B[w], int(RB[w]), sb, lb, tok_srcB)
    # replicate idx pattern across the 8 groups of 16 partitions
    idx_np[:, 16:, :] = np.tile(idx_np[:, :16, :], (1, 7, 1))

    # per-core node-level arrays in relabeled order
    inv = np.empty(N, np.int64)
    inv[new_id] = np.arange(N)

    nb = np.asarray(batch, dtype=np.int64)
    cnt = np.bincount(nb, minlength=G).astype(np.float64)

    plan = {
        "deg": deg,
        "dinv": dinv,
        "new_id": new_id,
        "inv": inv,
        "RA": RA,
        "RB": RB,
        "totch": totch,
        "idx": idx_np,
        "selT": selT_np,
        "cnt": cnt,
        "batch_new": nb[inv],  # graph id per relabeled node
        "tok_srcA": tok_srcA,
        "tok_srcB": tok_srcB,
    }
    return plan


# ------------------------------------------------------------- device build


def _build_nc(RA, RB, totch):
    import concourse.bacc as bacc
    import concourse.mybir as mybir
    import concourse.tile as tile

    fp32 = mybir.dt.float32
    bf16 = mybir.dt.bfloat16
    fp8 = mybir.dt.float8e4
    i16 = mybir.dt.int16
    DR = mybir.MatmulPerfMode.DoubleRow

    nc = bacc.Bacc("TRN2", debug=False, num_swdge_queues=4)

    tok0 = nc.dram_tensor("tok0", [128, totch, F], fp8, kind="ExternalInput")
    idxs = nc.dram_tensor("idxs", [128, totch * 8], i16, kind="ExternalInput")
    sels = nc.dram_tensor("sels", [128, totch, 128], fp8, kind="ExternalInput")
    wall0 = nc.dram_tensor("wall0", [F, EXP * HID], bf16, kind="ExternalInput")
    wall1 = nc.dram_tensor("wall1", [F, EXP * HID], bf16, kind="ExternalInput")
    ball0 = nc.dram_tensor("ball0", [1, EXP * HID], bf16, kind="ExternalInput")
    ball1 = nc.dram_tensor("ball1", [1, EXP * HID], bf16, kind="ExternalInput")
    sqdeg = nc.dram_tensor("sqdeg", [1, NPAD], bf16, kind="ExternalInput")
    dpow1 = nc.dram_tensor("dpow1", [128, WPC], fp32, kind="ExternalInput")
    dpow2 = nc.dram_tensor("dpow2", [128, WPC], fp32, kind="ExternalInput")
    topt = nc.dram_tensor("topt", [TOP, NPAD], bf16, kind="ExternalInput")
    wgt0 = nc.dram_tensor("wgt0", [TOP, EXP], bf16, kind="ExternalInput")
    wgt1 = nc.dram_tensor("wgt1", [TOP, EXP], bf16, kind="ExternalInput")
    h2out = nc.dram_tensor("h2out", [NSH, HID], bf16, kind="ExternalOutput")

    shard_a = nc.dram_tensor("shard_a", [PA, F], bf16)
    shard_b = nc.dram_tensor("shard_b", [PB, F], bf16)
    full1a = nc.dram_tensor("full1a", [W_CORES * PA, F], bf16,
                            addr_space="Shared")
    full1b = nc.dram_tensor("full1b", [W_CORES * PB, F], bf16,
                            addr_space="Shared")

    with tile.TileContext(nc) as tc:
        with tc.tile_pool(name="persist", bufs=1) as pp, \
             tc.tile_pool(name="wt", bufs=1) as wtp, \
             tc.tile_pool(name="stream", bufs=2) as sp, \
             tc.tile_pool(name="chunks", bufs=3) as cp, \
             tc.tile_pool(name="gatha", bufs=5) as gpa, \
             tc.tile_pool(name="gathb", bufs=4) as gpb, \
             tc.tile_pool(name="psum", bufs=4, space="PSUM") as ps, \
             tc.tile_pool(name="psume", bufs=3, space="PSUM") as pse:

            # ---------- resident data
            idx_sb = pp.tile([128, totch * 8], i16)
            nc.sync.dma_start(out=idx_sb[:], in_=idxs[:])
            hagg = pp.tile([128, NPAD], bf16)          # haggT, f-major
            sq_sb = pp.tile([1, NPAD], bf16)
            nc.sync.dma_start(out=sq_sb[:], in_=sqdeg[:])
            topt_sb = pp.tile([TOP, NPAD], bf16)
            nc.sync.dma_start(out=topt_sb[:], in_=topt[:])
            w_sb = [wtp.tile([F, EXP * HID], bf16, tag=f"w{l}", name=f"w{l}") for l in range(2)]
            nc.sync.dma_start(out=w_sb[0][:], in_=wall0[:])
            nc.sync.dma_start(out=w_sb[1][:], in_=wall1[:])
            b_sb = [wtp.tile([1, EXP * HID], bf16, tag=f"b{l}", name=f"b{l}") for l in range(2)]
            nc.sync.dma_start(out=b_sb[0][:], in_=ball0[:])
            nc.sync.dma_start(out=b_sb[1][:], in_=ball1[:])
            wg_sb = [wtp.tile([TOP, EXP], bf16, tag=f"wg{l}", name=f"wg{l}") for l in range(2)]
            nc.sync.dma_start(out=wg_sb[0][:], in_=wgt0[:])
            nc.sync.dma_start(out=wg_sb[1][:], in_=wgt1[:])
            dp_sb = [wtp.tile([128, WPC], fp32, tag=f"dp{l}", name=f"dp{l}") for l in range(2)]
            nc.sync.dma_start(out=dp_sb[0][:], in_=dpow1[:])
            nc.sync.dma_start(out=dp_sb[1][:], in_=dpow2[:])

            # chunk offsets in storage/call order
            chA, chB = {}, {}
            ch = 0
            for grp in GROUPS:
                for w in grp:
                    chA[w] = ch
                    ch += int(RA[w])
                for w in grp:
                    chB[w] = ch
                    ch += int(RB[w])

            scale_sb = []

            def compute_gates():
                # gate scale columns for both layers: [128, WPC, EXP]
                for l in range(2):
                    glog = pp.tile([128, WPC, EXP], fp32, tag=f"glog{l}", name=f"glog{l}")
                    for k in range(WPC):
                        pg = ps.tile([128, EXP], fp32, space="PSUM", tag="pw")
                        nc.tensor.matmul(
                            out=pg[:],
                            lhsT=topt_sb[:, k * 128 : (k + 1) * 128],
                            rhs=wg_sb[l][:],
                            start=True, stop=True,
                        )
                        nc.vector.tensor_copy(out=glog[:, k, :], in_=pg[:])
                    gexp = pp.tile([128, WPC, EXP], fp32, tag=f"gexp{l}", name=f"gexp{l}")
                    nc.scalar.activation(gexp[:], glog[:],
                                         mybir.ActivationFunctionType.Exp,
                                         bias=0.0, scale=1.0 / TEMP)
                    gsum = pp.tile([128, WPC], fp32, tag=f"gsum{l}", name=f"gsum{l}")
                    nc.vector.tensor_reduce(out=gsum[:], in_=gexp[:],
                                            axis=mybir.AxisListType.X,
                                            op=mybir.AluOpType.add)
                    grec = pp.tile([128, WPC], fp32, tag=f"grec{l}", name=f"grec{l}")
                    nc.vector.reciprocal(out=grec[:], in_=gsum[:])
                    rd = pp.tile([128, WPC], fp32, tag=f"rd{l}", name=f"rd{l}")
                    nc.vector.tensor_mul(out=rd[:], in0=grec[:], in1=dp_sb[l][:])
                    sc = pp.tile([128, WPC, EXP], fp32, tag=f"sc{l}", name=f"sc{l}")
                    for e in range(EXP):
                        nc.vector.tensor_mul(out=sc[:, :, e], in0=gexp[:, :, e],
                                             in1=rd[:])
                    scale_sb.append(sc)

            qrr = [0]

            def gather_calls(gtile, src_ap, ch0, nch):
                # 8-chunk single-packet SWDGE calls, round-robin queues
                off = 0
                while off < nch:
                    n = min(8, nch - off)
                    nc.gpsimd.dma_gather(
                        gtile[:, off : off + n, :], src_ap,
                        idx_sb[:, (ch0 + off) * 8 : (ch0 + off + n) * 8],
                        n * 128, n * 128, F, single_packet=True,
                        queue_num=qrr[0] % 4)
                    qrr[0] += 1
                    off += n

            PF = 4  # A-bucket gather prefetch depth (groups)

            def issue_gA(grp):
                ra = sum(int(RA[w]) for w in grp)
                gA = gpa.tile([128, ra, F], bf16, tag="gA")
                gather_calls(gA, full1a[:], chA[grp[0]], ra)
                return gA

            def sparse_and_dense(l, store_l1):
                if l == 0:
                    compute_gates()
                # layer 2 iterates the tiny single-window group first so
                # the critical post-AllGather B-bucket gather is 1 call
                giter = list(GROUPS) if l == 0 else \
                    [GROUPS[-1]] + list(GROUPS[:-1])
                gA_pend = {}
                gB0 = None
                if l == 1:
                    # issue one piece-1 gather first: it waits for the
                    # second AllGather, serializing every later gather
                    # behind it on the gpsimd queue so the collectives
                    # run without DMA contention.
                    grp0 = giter[0]
                    rb0 = sum(int(RB[w]) for w in grp0)
                    gB0 = gpb.tile([128, rb0, F], bf16, tag="gB")
                    gather_calls(gB0, full1b[:], chB[grp0[0]], rb0)
                    for gi in range(min(PF, len(giter))):
                        gA_pend[gi] = issue_gA(giter[gi])
                for gidx, grp in enumerate(giter):
                    ra = sum(int(RA[w]) for w in grp)
                    rb = sum(int(RB[w]) for w in grp)
                    c0 = chA[grp[0]]
                    cb0 = chB[grp[0]]
                    selAll = cp.tile([128, ra + rb, 128], fp8, tag="selAll")
                    nc.sync.dma_start(
                        out=selAll[:], in_=sels[:, c0 : c0 + ra + rb, :])
                    if l == 0:
                        gAll = cp.tile([128, ra + rb, F], fp8, tag="gAll")
                        nc.sync.dma_start(
                            out=gAll[:], in_=tok0[:, c0 : c0 + ra + rb, :])
                        gA = gAll
                        gB = gAll
                        boff = 0          # B chunks at [ra, ra+rb) in gAll
                    else:
                        gA = gA_pend.pop(gidx)
                        if gidx + PF < len(giter):
                            gA_pend[gidx + PF] = issue_gA(giter[gidx + PF])
                        if gidx == 0:
                            gB = gB0
                        else:
                            gB = gpb.tile([128, rb, F], bf16, tag="gB")
                            gather_calls(gB, full1b[:], cb0, rb)
                        boff = -ra        # gB chunk axis starts at 0
                    a_off = 0
                    b_off = 0
                    for w in grp:
                        pw = ps.tile([128, WSLOT], fp32, space="PSUM", tag="pw")
                        # runs: (token tile, tile_off - sel_off, sel chunk, count)
                        nmm_plan = []
                        for tile_, toff, selc, cnt in [
                            (gA, 0, a_off, int(RA[w])),
                            (gB, boff, ra + b_off, int(RB[w])),
                        ]:
                            r = 0
                            while r < cnt:
                                if l == 0 and r + 1 < cnt:
                                    nmm_plan.append((tile_, toff, selc + r, 2))
                                    r += 2
                                else:
                                    nmm_plan.append((tile_, toff, selc + r, 1))
                                    r += 1
                        nmm = len(nmm_plan)
                        for j, (tile_, toff, selc, k) in enumerate(nmm_plan):
                            tc0 = selc + toff
                            if k == 2:
                                nc.tensor.matmul(
                                    out=pw[:],
                                    lhsT=tile_[:, tc0 : tc0 + 2, :],
                                    rhs=selAll[:, selc : selc + 2, :],
                                    start=(j == 0), stop=(j == nmm - 1),
                                    perf_mode=DR)
                            else:
                                nc.tensor.matmul(
                                    out=pw[:],
                                    lhsT=tile_[:, tc0, :],
                                    rhs=selAll[:, selc, :],
                                    start=(j == 0), stop=(j == nmm - 1))
                        a_off += int(RA[w])
                        b_off += int(RB[w])
                        nc.vector.tensor_copy(
                            out=hagg[:, w * 128 : (w + 1) * 128], in_=pw[:])
                    # dense for this group's windows, interleaved so the
                    # PE fills gather-wait gaps and (layer 1) the piece-0
                    # AllGather fires mid-way through the sparse stream
                    for k in grp:
                        dense_window(l, k, store_l1)

            def dense_window(l, k, store_l1):
                pe = pse.tile([128, EXP * HID], fp32, space="PSUM", tag="pe")
                nc.tensor.matmul(
                    out=pe[:], lhsT=hagg[:, k * 128 : (k + 1) * 128],
                    rhs=w_sb[l][:], start=True, stop=False)
                nc.tensor.matmul(
                    out=pe[:], lhsT=sq_sb[:, k * 128 : (k + 1) * 128],
                    rhs=b_sb[l][:], start=False, stop=True)
                # experts 0,1 on the scalar engine; expert 2 on DVE
                aex = []
                for e in range(2):
                    a = cp.tile([128, HID], fp32, tag=f"a{e}", name=f"a{e}")
                    nc.scalar.activation(
                        a[:], pe[:, e * HID : (e + 1) * HID],
                        mybir.ActivationFunctionType.Relu,
                        bias=0.0, scale=scale_sb[l][:, k, e : e + 1])
                    aex.append(a)
                a2 = cp.tile([128, HID], fp32, tag="a2", name="a2")
                nc.vector.tensor_scalar(
                    out=a2[:], in0=pe[:, 2 * HID : 3 * HID],
                    scalar1=scale_sb[l][:, k, 2 : 3], scalar2=0.0,
                    op0=mybir.AluOpType.mult, op1=mybir.AluOpType.max)
                hout = cp.tile([128, HID], fp32, tag="hout")
                nc.vector.tensor_add(out=hout[:], in0=aex[0][:], in1=aex[1][:])
                hbf = cp.tile([128, HID], bf16, tag="hbf")
                nc.vector.tensor_add(out=hbf[:], in0=hout[:], in1=a2[:])
                rows = min(128, NSH - k * 128)
                if store_l1:
                    if k < WSPLIT:
                        nc.sync.dma_start(
                            out=shard_a[k * 128 : k * 128 + rows, :],
                            in_=hbf[:rows, :])
                    else:
                        r0 = (k - WSPLIT) * 128
                        nc.sync.dma_start(
                            out=shard_b[r0 : r0 + rows, :],
                            in_=hbf[:rows, :])
                    if k == WSPLIT - 1:
                        nc.gpsimd.collective_compute(
                            "AllGather", mybir.AluOpType.bypass,
                            ins=[shard_a[:]], outs=[full1a[:]],
                            replica_groups=[list(range(W_CORES))])
                else:
                    nc.sync.dma_start(
                        out=h2out[k * 128 : k * 128 + rows, :],
                        in_=hbf[:rows, :])

            # ---------- layer 1 (piece-0 AllGather fires inside its dense loop)
            sparse_and_dense(0, store_l1=True)
            nc.gpsimd.collective_compute(
                "AllGather", mybir.AluOpType.bypass,
                ins=[shard_b[:]], outs=[full1b[:]],
                replica_groups=[list(range(W_CORES))])
            # ---------- layer 2 (h2 rows stream out; pooling on host)
            sparse_and_dense(1, store_l1=False)

    nc.compile()
    return nc


# ------------------------------------------------------------------- kernel


def kernel(**inputs):
    x = np.asarray(inputs["x"], np.float32)
    top_features = np.asarray(inputs["top_features"], np.float32)
    edge_index = np.asarray(inputs["edge_index"])
    batch = np.asarray(inputs["batch"])
    W0 = np.asarray(inputs["W0"], np.float32)
    b0 = np.asarray(inputs["b0"], np.float32)
    Wg0 = np.asarray(inputs["Wg0"], np.float32)
    W1 = np.asarray(inputs["W1"], np.float32)
    b1 = np.asarray(inputs["b1"], np.float32)
    Wg1 = np.asarray(inputs["Wg1"], np.float32)
    Wf = np.asarray(inputs["Wf"], np.float32)
    bf = np.asarray(inputs["bf"], np.float32)

    plan = _build_plan(edge_index, batch)
    dinv = plan["dinv"]
    inv = plan["inv"]          # relabeled -> original node id
    RA, RB, totch = plan["RA"], plan["RB"], plan["totch"]

    # gather source (layer 1): x * dinv, relabeled order, fp8
    xs8 = (x * dinv[:, None])[inv].astype(ml_dtypes.float8_e4m3)

    deg_new = plan["deg"][inv]
    dinv_new = dinv[inv]
    top_new = top_features[inv]
    batch_new = plan["batch_new"]

    def pad_npad(a):
        out = np.zeros((W_CORES, NPAD) + a.shape[1:], a.dtype)
        for c in range(W_CORES):
            out[c, : 48 * WSLOT] = a[c * NSH : c * NSH + 48 * WSLOT]
            # last window: 106 real slots
            out[c, 48 * WSLOT : 48 * WSLOT + (NSH - 48 * WSLOT)] = \
                a[c * NSH + 48 * WSLOT : (c + 1) * NSH]
        return out

    sq_pad = pad_npad(np.sqrt(deg_new).astype(np.float32))       # [8, NPAD]
    d1_pad = pad_npad((dinv_new ** 2).astype(np.float32))
    d2_pad = pad_npad(dinv_new.astype(np.float32))
    top_pad = pad_npad(top_new.astype(np.float32))               # [8,NPAD,4]
    bat_pad = pad_npad(batch_new)
    # mark pad slots: zero scales, selg zero
    padmask = pad_npad(np.ones(N, np.float32))

    d1_pad *= padmask
    d2_pad *= padmask

    wall0 = W0.transpose(1, 0, 2).reshape(F, EXP * HID).copy()
    wall1 = W1.transpose(1, 0, 2).reshape(F, EXP * HID).copy()
    ball0 = b0.reshape(1, EXP * HID).copy()
    ball1 = b1.reshape(1, EXP * HID).copy()

    # relabeled-node -> tok0 source row (both pieces share xs8 order)
    # piece-local row -> relabeled id: A: c*PA + loc ; B: c*PB + loc - ...
    in_maps = []
    for c in range(W_CORES):
        tsA = plan["tok_srcA"][c]
        tsB = plan["tok_srcB"][c]
        tok0_c = np.zeros((totch, 128, F), ml_dtypes.float8_e4m3)
        va = tsA >= 0
        if va.any():
            ra = tsA[va]
            rel = (ra // PA) * NSH + (ra % PA)
            tok0_c[va] = xs8[rel]
        vb = tsB >= 0
        if vb.any():
            rb = tsB[vb]
            rel = (rb // PB) * NSH + PA + (rb % PB)
            tok0_c[vb] = xs8[rel]
        tok0T_c = np.ascontiguousarray(tok0_c.transpose(1, 0, 2))
        in_maps.append({
            "tok0": tok0T_c,
            "idxs": plan["idx"][c],
            "sels": np.ascontiguousarray(plan["selT"][c]),
            "wall0": wall0.astype(ml_dtypes.bfloat16),
            "wall1": wall1.astype(ml_dtypes.bfloat16),
            "ball0": ball0.astype(ml_dtypes.bfloat16),
            "ball1": ball1.astype(ml_dtypes.bfloat16),
            "sqdeg": sq_pad[c][None, :].astype(ml_dtypes.bfloat16),
            "dpow1": d1_pad[c].reshape(WPC, 128).T.copy(),
            "dpow2": d2_pad[c].reshape(WPC, 128).T.copy(),
            "topt": top_pad[c].T.copy().astype(ml_dtypes.bfloat16),
            "wgt0": Wg0.T.copy().astype(ml_dtypes.bfloat16),
            "wgt1": Wg1.T.copy().astype(ml_dtypes.bfloat16),
        })

    from concourse.bass_utils import run_bass_kernel_spmd

    nc = _build_nc(RA, RB, totch)
    trace = os.environ.get("KERNEL_TRACE", "0") == "1"
    ncores = int(os.environ.get("KERNEL_CORES", str(W_CORES)))
    res = run_bass_kernel_spmd(nc, in_maps[:ncores], core_ids=list(range(ncores)),
                               trace=trace)
    kernel.last_results = res

    # host-side pooling: segment-mean over graphs + final linear layer
    h2 = np.concatenate(
        [np.asarray(res.results[c]["h2out"]).astype(np.float64)
         for c in range(W_CORES)], axis=0)                    # [N, HID] relab
    sums = np.zeros((G, HID), np.float64)
    np.add.at(sums, batch_new, h2)
    cnt = np.maximum(plan["cnt"], 1.0)
    pooled = sums / cnt[:, None]
    out = pooled @ Wf.astype(np.float64) + bf.astype(np.float64)[None, :]
    return out.astype(np.float32)
